# revision 27
# baseline (speedup 1.0000x reference)
"""Trainium2 Bass kernel for nn_AttentionBlock (b,h,w,c = 32,64,64,256).

out = x + (softmax_w(QK^T * s) @ V) @ Wo + bo   with Q/K/V = x@W* + b*
per-row attention over the w axis, batch-parallel over 8 NeuronCores.

Algebra used by the kernel (validated against the jax reference):
  scores*s = x A x^T + ones_i (x wv)^T   where A = (Wq Wk^T) s, wv = (Wk bq) s
  (the i-side bias term and the constant cancel inside softmax)
  out = attn@V@Wo + (bv@Wo + bo) + x
Compute dtype: bf16 operands into the PE array, fp32 PSUM accumulation,
fp32 residual add. Inputs/outputs stay fp32.
"""

import os
import sys

for _p in ("/opt/trn_rl_repo", os.path.expanduser("~/.axon_site/_ro/trn_rl_repo")):
    if os.path.isdir(_p) and _p not in sys.path:
        sys.path.append(_p)

import numpy as np

import concourse.bass as bass
import concourse.mybir as mybir
import concourse.tile as tile
from concourse import bacc
from concourse.masks import make_identity

N_CORES = 8
B, H, W, C = 32, 64, 64, 256
BPC = B // N_CORES            # batch images per core
RPC = BPC * H * W             # rows per core = 16384
SCALE = 1.0 / (C * np.sqrt(0.5) * np.sqrt(C))   # folded softmax scale

F32 = mybir.dt.float32
BF16 = mybir.dt.bfloat16

CHUNK = 2048                  # rows per chunk (32 attention pairs)
N_RT = CHUNK // 128           # row-tiles per chunk
N_PAIR = CHUNK // W           # pairs per chunk
N_GRP = N_PAIR // 8           # 8-pair score groups per chunk


def _build_body(nc, tc, x_d, w_d, b_d, out_d, n_chunks, ctx, use_bias):
    """Emit the kernel body. w_d/b_d: dicts of weight/bias dram handles."""

    def pool(name, bufs, space="SBUF"):
        kw = {} if space == "SBUF" else {"space": bass.MemorySpace.PSUM}
        return ctx.enter_context(tc.tile_pool(name=name, bufs=bufs, **kw))

    const = pool("const", 1)
    wtmp = pool("wtmp", 2)
    ptx = pool("ptx", 2, "PSUM")      # [128,128] bf16: transposes
    pbig = pool("pbig", 2, "PSUM")    # [128,256] f32: V, oU, A
    psc = pool("psc", 2, "PSUM")      # [128,4,128] f32: scores^T super-tiles
    ppo = pool("ppo", 1, "PSUM")      # [128,256] f32: O projection
    pgv = pool("pgv", 1, "PSUM")      # [128,512] f32: G blocks, xv

    # ---------------- preamble: constants & weight prep ----------------
    ident_b = const.tile([128, 128], BF16, tag="identb")
    make_identity(nc, ident_b)
    ones_b = const.tile([1, 128], BF16, tag="ones")
    nc.vector.memset(ones_b, 1.0)
    onescol_b = const.tile([128, 1], BF16, tag="onescol")
    nc.vector.memset(onescol_b, 1.0)

    # transposed Wq/Wk/Wv: WT[:, dc, c] = W[c, 128*dc + d]
    WqT = const.tile([128, 2, 256], BF16, tag="wqt")
    WkT = const.tile([128, 2, 256], BF16, tag="wkt")
    WvT = const.tile([128, 2, 256], BF16, tag="wvt")
    Wo_b = const.tile([128, 2, 256], BF16, tag="wob")
    for wname, wt in (("Wq", WqT), ("Wk", WkT), ("Wv", WvT)):
        for cc in range(2):
            wrow = wtmp.tile([128, 256], F32, tag="wrow")
            nc.sync.dma_start(out=wrow, in_=w_d[wname][cc * 128:(cc + 1) * 128, :])
            wrow_b = wtmp.tile([128, 256], BF16, tag="wrowb")
            nc.vector.tensor_copy(wrow_b, wrow)
            for dc in range(2):
                tp = ptx.tile([128, 128], BF16, tag="tx")
                nc.tensor.transpose(tp, wrow_b[:, dc * 128:(dc + 1) * 128], ident_b)
                nc.any.tensor_copy(wt[:, dc, cc * 128:(cc + 1) * 128], tp)
    for wname, wt in (("Wo", Wo_b),):
        for cc in range(2):
            wrow = wtmp.tile([128, 256], F32, tag="wrow")
            nc.sync.dma_start(out=wrow, in_=w_d[wname][cc * 128:(cc + 1) * 128, :])
            nc.any.tensor_copy(wt[:, cc, :], wrow)

    # A[c, a] = s * sum_d Wq[c, d] Wk[a, d]
    A_b = const.tile([128, 2, 256], BF16, tag="ab")
    for cc in range(2):
        pa = pbig.tile([128, 256], F32, tag="m")
        for dc in range(2):
            nc.tensor.matmul(pa, WqT[:, dc, cc * 128:(cc + 1) * 128], WkT[:, dc, :],
                             start=(dc == 0), stop=(dc == 1))
        nc.any.tensor_scalar_mul(A_b[:, cc, :], pa, float(SCALE))

    # U[c, e] = sum_d Wv[c, d] Wo[d, e]  (fused V+output projection)
    U_b = const.tile([128, 2, 256], BF16, tag="ub")
    for cc in range(2):
        pu = pbig.tile([128, 256], F32, tag="m")
        for dc in range(2):
            nc.tensor.matmul(pu, WvT[:, dc, cc * 128:(cc + 1) * 128],
                             Wo_b[:, dc, :], start=(dc == 0), stop=(dc == 1))
        nc.any.tensor_copy(U_b[:, cc, :], pu)

    wv_b = bo2_b = None
    if use_bias:
        # bias columns
        bq_b = const.tile([128, 2], BF16, tag="bqb")
        bv_b = const.tile([128, 2], BF16, tag="bvb")
        for bname, bt in (("bq", bq_b), ("bv", bv_b)):
            bf = wtmp.tile([128, 2], F32, tag="bcol")
            for cc in range(2):
                nc.sync.dma_start(out=bf[:, cc:cc + 1],
                                  in_=b_d[bname][cc * 128:(cc + 1) * 128].unsqueeze(1))
            nc.any.tensor_copy(bt, bf)

        # wv[c] = s * sum_d Wk[c, d] bq[d]
        wv_b = const.tile([128, 2], BF16, tag="wvvec")
        for cc in range(2):
            pw = psc.tile([128, 128], F32, tag="sc")
            for dc in range(2):
                nc.tensor.matmul(pw[:, 0:1], WkT[:, dc, cc * 128:(cc + 1) * 128],
                                 bq_b[:, dc:dc + 1], start=(dc == 0), stop=(dc == 1))
            nc.any.tensor_scalar_mul(wv_b[:, cc:cc + 1], pw[:, 0:1], float(SCALE))

        # bo2[e] = bv @ Wo + bo  (fused output bias, bf16 [1,256] row)
        bo_f = wtmp.tile([1, 256], F32, tag="borow")
        nc.sync.dma_start(out=bo_f, in_=b_d["bo"][:].unsqueeze(0))
        pb = pbig.tile([128, 256], F32, tag="m")
        for cc in range(2):
            nc.tensor.matmul(pb[0:1, :], bv_b[:, cc:cc + 1], Wo_b[:, cc, :],
                             start=(cc == 0), stop=(cc == 1))
        bo2_f = wtmp.tile([1, 256], F32, tag="bo2row")
        nc.vector.tensor_add(bo2_f, pb[0:1, :], bo_f)
        bo2_dram = nc.dram_tensor("bo2_bounce", [256], F32)
        nc.sync.dma_start(out=bo2_dram[:].unsqueeze(0), in_=bo2_f)
        bo2_bc = const.tile([128, 256], F32, tag="bo2bc")
        bo2_src = bass.AP(tensor=bo2_dram, offset=0,
                          ap=[[0, 128], [1, 256]])
        nc.sync.dma_start(out=bo2_bc, in_=bo2_src)

    # ---------------- main loop ----------------
    xpool = pool("x", 2)
    xbpool = pool("xb", 3)
    expool = pool("expt", 3)
    rspool = pool("rs", 3)
    xtpool = pool("xt", 2)
    gtpool = pool("gt", 2)
    vpool = pool("v", 2)
    xvpool = pool("xv", 2)
    opool = pool("o", 3)
    ppool = pool("p", 2)
    outpool = pool("outs", 3)

    for ch in range(n_chunks):
        r0 = ch * CHUNK
        x_f = xpool.tile([128, N_RT, 256], F32, tag="xf")
        xT = xtpool.tile([128, 2, CHUNK], BF16, tag="xt")
        GT = gtpool.tile([128, 2, CHUNK], BF16, tag="gt")
        xb_all = vpool.tile([128, N_RT, 256], BF16, tag="xball")
        xv_b = xvpool.tile([1, CHUNK], BF16, tag="xvb")

        # load x, cast to bf16 (kept for the Y stage), transpose on PE
        for rt in range(N_RT):
            rr = r0 + rt * 128
            nc.sync.dma_start(out=x_f[:, rt, :], in_=x_d[rr:rr + 128, :])
            nc.vector.tensor_copy(xb_all[:, rt, :], x_f[:, rt, :])
            tpx = ptx.tile([128, 2, 128], BF16, tag="tx")
            for cc in range(2):
                nc.tensor.transpose(tpx[:, cc, :],
                                    xb_all[:, rt, cc * 128:(cc + 1) * 128],
                                    ident_b)
            nc.any.tensor_copy(xT[:, :, rt * 128:(rt + 1) * 128], tpx)

        # G^T = A^T x^T (a on partitions), xv = x . wv, in 512-col blocks
        for blk in range(CHUNK // 512):
            cs = blk * 512
            for ac in range(2):
                pg = pgv.tile([128, 512], F32, tag="g")
                for cc in range(2):
                    nc.tensor.matmul(pg, A_b[:, cc, ac * 128:(ac + 1) * 128],
                                     xT[:, cc, cs:cs + 512],
                                     start=(cc == 0), stop=(cc == 1))
                nc.any.tensor_copy(GT[:, ac, cs:cs + 512], pg)
            if use_bias:
                pxv = pgv.tile([128, 512], F32, tag="g")
                for cc in range(2):
                    nc.tensor.matmul(pxv[0:1, :], wv_b[:, cc:cc + 1],
                                     xT[:, cc, cs:cs + 512],
                                     start=(cc == 0), stop=(cc == 1))
                nc.any.tensor_copy(xv_b[0:1, cs:cs + 512], pxv[0:1, :])

        # attention, TRANSPOSED scores, 4 row-tiles per PSUM super-tile:
        # scT[j, i] = x_j . G_i per 128-row window; diagonal 64x64 blocks
        # are the pairs, off-diagonal cross-pair garbage. All 4 windows'
        # matmuls form ONE psum accumulation group (one bank). exp() runs
        # on the diagonal blocks of all 4 windows in one ACT op per half,
        # into a zeroed bf16 tile. 1/rowsum folds into the final residual.
        for sg in range(N_RT // 4):
            scT4 = psc.tile([128, 4, 128], F32, tag="sc")
            nmm = 12 if use_bias else 8
            mi = 0
            for r in range(4):
                rt = sg * 4 + r
                ir = rt * 128
                for ac in range(2):
                    nc.tensor.matmul(scT4[:, r, :], xT[:, ac, ir:ir + 128],
                                     GT[:, ac, ir:ir + 128],
                                     start=(mi == 0), stop=(mi == nmm - 1))
                    mi += 1
                if use_bias:
                    nc.tensor.matmul(scT4[:, r, :], xv_b[0:1, ir:ir + 128],
                                     ones_b, start=False, stop=(mi == nmm - 1))
                    mi += 1
            expT4 = expool.tile([128, 4, 128], BF16, tag="expt")
            nc.gpsimd.memset(expT4, 0.0)
            for il in range(2):
                dg = slice(il * 64, (il + 1) * 64)
                cg = slice(il * 64, il * 64 + 64)
                nc.scalar.activation(expT4[dg, :, cg], scT4[dg, :, cg],
                                     mybir.ActivationFunctionType.Exp)
            # rowsums over j for all 4 windows -> [i, 4], one reciprocal
            prs4 = ppo.tile([128, 4], F32, tag="po")
            for r in range(4):
                nc.tensor.matmul(prs4[:, r:r + 1], expT4[:, r, :], onescol_b,
                                 start=(r == 0), stop=(r == 3))
            rrs4 = rspool.tile([128, 4], F32, tag="rrs")
            nc.vector.reciprocal(rrs4, prs4)
            for r in range(4):
                rt = sg * 4 + r
                # Y[c, i] = sum_j x[j, c] p^T[j, i]  (V projection fused
                # into U = Wv @ Wo on the output side)
                pou = pbig.tile([128, 256], F32, tag="m")
                for cc in range(2):
                    nc.tensor.matmul(pou[:, cc * 128:(cc + 1) * 128],
                                     xb_all[:, rt, cc * 128:(cc + 1) * 128],
                                     expT4[:, r, :], start=True, stop=True)
                Y_sb = opool.tile([128, 2, 128], BF16, tag="out_t")
                nc.any.tensor_copy(Y_sb, pou.rearrange("p (c i) -> p c i", c=2))
                # final projection, normalization, residual
                pO = ppo.tile([128, 256], F32, tag="po")
                nc.tensor.matmul(pO, Y_sb[:, 0, :], U_b[:, 0, :],
                                 start=True, stop=False)
                nc.tensor.matmul(pO, Y_sb[:, 1, :], U_b[:, 1, :],
                                 start=False, stop=True)
                o_sb = outpool.tile([128, 256], F32, tag="osb")
                nc.vector.scalar_tensor_tensor(
                    o_sb, pO, rrs4[:, r:r + 1], x_f[:, rt, :],
                    op0=mybir.AluOpType.mult, op1=mybir.AluOpType.add)
                if use_bias:
                    nc.vector.tensor_add(o_sb, o_sb, bo2_bc)
                rr = r0 + rt * 128
                nc.sync.dma_start(out=out_d[rr:rr + 128, :], in_=o_sb)



def build(n_chunks=RPC // CHUNK, use_bias=True):
    nc = bacc.Bacc("TRN2", target_bir_lowering=False, debug=False)
    rows = n_chunks * CHUNK
    x_d = nc.declare_dram_parameter("x", [rows, C], F32, isOutput=False)
    w_d = {n: nc.declare_dram_parameter(n, [C, C], F32, isOutput=False)
           for n in ("Wq", "Wk", "Wv", "Wo")}
    b_d = {n: nc.declare_dram_parameter(n, [C], F32, isOutput=False)
           for n in ("bq", "bk", "bv", "bo")}
    out_d = nc.declare_dram_parameter("out", [rows, C], F32, isOutput=True)
    from contextlib import ExitStack
    with tile.TileContext(nc) as tc, ExitStack() as ctx:
        _build_body(nc, tc, x_d, w_d, b_d, out_d, n_chunks, ctx, use_bias)
    nc.compile()
    return nc


_NC = {}
TRACE = False
LAST_RESULT = None


def kernel(x, Wq, bq, Wk, bk, Wv, bv, Wo, bo):
    global LAST_RESULT
    use_bias = any(np.any(np.asarray(b)) for b in (bq, bk, bv, bo))
    if use_bias not in _NC:
        _NC[use_bias] = build(use_bias=use_bias)
    nc_k = _NC[use_bias]
    from concourse.bass_utils import run_bass_kernel_spmd

    x = np.ascontiguousarray(np.asarray(x, dtype=np.float32))
    shared = {
        "Wq": np.ascontiguousarray(Wq, dtype=np.float32),
        "Wk": np.ascontiguousarray(Wk, dtype=np.float32),
        "Wv": np.ascontiguousarray(Wv, dtype=np.float32),
        "Wo": np.ascontiguousarray(Wo, dtype=np.float32),
        "bq": np.ascontiguousarray(bq, dtype=np.float32),
        "bk": np.ascontiguousarray(bk, dtype=np.float32),
        "bv": np.ascontiguousarray(bv, dtype=np.float32),
        "bo": np.ascontiguousarray(bo, dtype=np.float32),
    }
    in_maps = []
    for i in range(N_CORES):
        xs = np.ascontiguousarray(
            x[i * BPC:(i + 1) * BPC].reshape(RPC, C))
        in_maps.append({"x": xs, **shared})
    res = run_bass_kernel_spmd(nc_k, in_maps, core_ids=list(range(N_CORES)),
                               trace=TRACE)
    LAST_RESULT = res
    out = np.concatenate(
        [res.results[i]["out"].reshape(BPC, H, W, C) for i in range(N_CORES)],
        axis=0)
    return out


# revision 28
# speedup vs baseline: 1.0306x; 1.0306x over previous
"""Trainium2 Bass kernel for nn_AttentionBlock (b,h,w,c = 32,64,64,256).

out = x + (softmax_w(QK^T * s) @ V) @ Wo + bo   with Q/K/V = x@W* + b*
per-row attention over the w axis, batch-parallel over 8 NeuronCores.

Algebra used by the kernel (validated against the jax reference):
  scores*s = x A x^T + ones_i (x wv)^T   where A = (Wq Wk^T) s, wv = (Wk bq) s
  (the i-side bias term and the constant cancel inside softmax)
  out = attn@V@Wo + (bv@Wo + bo) + x
Compute dtype: bf16 operands into the PE array, fp32 PSUM accumulation,
fp32 residual add. Inputs/outputs stay fp32.
"""

import os
import sys

for _p in ("/opt/trn_rl_repo", os.path.expanduser("~/.axon_site/_ro/trn_rl_repo")):
    if os.path.isdir(_p) and _p not in sys.path:
        sys.path.append(_p)

import numpy as np

import concourse.bass as bass
import concourse.mybir as mybir
import concourse.tile as tile
from concourse import bacc
from concourse.masks import make_identity

N_CORES = 8
B, H, W, C = 32, 64, 64, 256
BPC = B // N_CORES            # batch images per core
RPC = BPC * H * W             # rows per core = 16384
SCALE = 1.0 / (C * np.sqrt(0.5) * np.sqrt(C))   # folded softmax scale

F32 = mybir.dt.float32
BF16 = mybir.dt.bfloat16

CHUNK = 2048                  # rows per chunk (32 attention pairs)
N_RT = CHUNK // 128           # row-tiles per chunk
N_PAIR = CHUNK // W           # pairs per chunk
N_GRP = N_PAIR // 8           # 8-pair score groups per chunk


def _build_body(nc, tc, x_d, w_d, b_d, out_d, n_chunks, ctx, use_bias):
    """Emit the kernel body. w_d/b_d: dicts of weight/bias dram handles."""

    def pool(name, bufs, space="SBUF"):
        kw = {} if space == "SBUF" else {"space": bass.MemorySpace.PSUM}
        return ctx.enter_context(tc.tile_pool(name=name, bufs=bufs, **kw))

    const = pool("const", 1)
    wtmp = pool("wtmp", 2)
    ptx = pool("ptx", 2, "PSUM")      # [128,128] bf16: transposes
    pbig = pool("pbig", 2, "PSUM")    # [128,256] f32: V, oU, A
    psc = pool("psc", 2, "PSUM")      # [128,4,128] f32: scores^T super-tiles
    ppo = pool("ppo", 1, "PSUM")      # [128,256] f32: O projection
    pgv = pool("pgv", 1, "PSUM")      # [128,512] f32: G blocks, xv

    # ---------------- preamble: constants & weight prep ----------------
    ident_b = const.tile([128, 128], BF16, tag="identb")
    make_identity(nc, ident_b)
    ones_b = const.tile([1, 128], BF16, tag="ones")
    nc.vector.memset(ones_b, 1.0)
    onescol_b = const.tile([128, 1], BF16, tag="onescol")
    nc.vector.memset(onescol_b, 1.0)

    # transposed Wq/Wk/Wv: WT[:, dc, c] = W[c, 128*dc + d]
    WqT = const.tile([128, 2, 256], BF16, tag="wqt")
    WkT = const.tile([128, 2, 256], BF16, tag="wkt")
    WvT = const.tile([128, 2, 256], BF16, tag="wvt")
    Wo_b = const.tile([128, 2, 256], BF16, tag="wob")
    for wname, wt in (("Wq", WqT), ("Wk", WkT), ("Wv", WvT)):
        for cc in range(2):
            wrow = wtmp.tile([128, 256], F32, tag="wrow")
            nc.sync.dma_start(out=wrow, in_=w_d[wname][cc * 128:(cc + 1) * 128, :])
            wrow_b = wtmp.tile([128, 256], BF16, tag="wrowb")
            nc.vector.tensor_copy(wrow_b, wrow)
            for dc in range(2):
                tp = ptx.tile([128, 128], BF16, tag="tx")
                nc.tensor.transpose(tp, wrow_b[:, dc * 128:(dc + 1) * 128], ident_b)
                nc.any.tensor_copy(wt[:, dc, cc * 128:(cc + 1) * 128], tp)
    for wname, wt in (("Wo", Wo_b),):
        for cc in range(2):
            wrow = wtmp.tile([128, 256], F32, tag="wrow")
            nc.sync.dma_start(out=wrow, in_=w_d[wname][cc * 128:(cc + 1) * 128, :])
            nc.any.tensor_copy(wt[:, cc, :], wrow)

    # A[c, a] = s * sum_d Wq[c, d] Wk[a, d]
    A_b = const.tile([128, 2, 256], BF16, tag="ab")
    for cc in range(2):
        pa = pbig.tile([128, 256], F32, tag="m")
        for dc in range(2):
            nc.tensor.matmul(pa, WqT[:, dc, cc * 128:(cc + 1) * 128], WkT[:, dc, :],
                             start=(dc == 0), stop=(dc == 1))
        nc.any.tensor_scalar_mul(A_b[:, cc, :], pa, float(SCALE))

    # U[c, e] = sum_d Wv[c, d] Wo[d, e]  (fused V+output projection)
    U_b = const.tile([128, 2, 256], BF16, tag="ub")
    for cc in range(2):
        pu = pbig.tile([128, 256], F32, tag="m")
        for dc in range(2):
            nc.tensor.matmul(pu, WvT[:, dc, cc * 128:(cc + 1) * 128],
                             Wo_b[:, dc, :], start=(dc == 0), stop=(dc == 1))
        nc.any.tensor_copy(U_b[:, cc, :], pu)

    wv_b = bo2_b = None
    if use_bias:
        # bias columns
        bq_b = const.tile([128, 2], BF16, tag="bqb")
        bv_b = const.tile([128, 2], BF16, tag="bvb")
        for bname, bt in (("bq", bq_b), ("bv", bv_b)):
            bf = wtmp.tile([128, 2], F32, tag="bcol")
            for cc in range(2):
                nc.sync.dma_start(out=bf[:, cc:cc + 1],
                                  in_=b_d[bname][cc * 128:(cc + 1) * 128].unsqueeze(1))
            nc.any.tensor_copy(bt, bf)

        # wv[c] = s * sum_d Wk[c, d] bq[d]
        wv_b = const.tile([128, 2], BF16, tag="wvvec")
        for cc in range(2):
            pw = psc.tile([128, 128], F32, tag="sc")
            for dc in range(2):
                nc.tensor.matmul(pw[:, 0:1], WkT[:, dc, cc * 128:(cc + 1) * 128],
                                 bq_b[:, dc:dc + 1], start=(dc == 0), stop=(dc == 1))
            nc.any.tensor_scalar_mul(wv_b[:, cc:cc + 1], pw[:, 0:1], float(SCALE))

        # bo2[e] = bv @ Wo + bo  (fused output bias, bf16 [1,256] row)
        bo_f = wtmp.tile([1, 256], F32, tag="borow")
        nc.sync.dma_start(out=bo_f, in_=b_d["bo"][:].unsqueeze(0))
        pb = pbig.tile([128, 256], F32, tag="m")
        for cc in range(2):
            nc.tensor.matmul(pb[0:1, :], bv_b[:, cc:cc + 1], Wo_b[:, cc, :],
                             start=(cc == 0), stop=(cc == 1))
        bo2_f = wtmp.tile([1, 256], F32, tag="bo2row")
        nc.vector.tensor_add(bo2_f, pb[0:1, :], bo_f)
        bo2_dram = nc.dram_tensor("bo2_bounce", [256], F32)
        nc.sync.dma_start(out=bo2_dram[:].unsqueeze(0), in_=bo2_f)
        bo2_bc = const.tile([128, 256], F32, tag="bo2bc")
        bo2_src = bass.AP(tensor=bo2_dram, offset=0,
                          ap=[[0, 128], [1, 256]])
        nc.sync.dma_start(out=bo2_bc, in_=bo2_src)

    # ---------------- main loop ----------------
    xpool = pool("x", 2)
    xbpool = pool("xb", 3)
    expool = pool("expt", 3)
    rspool = pool("rs", 3)
    xtpool = pool("xt", 2)
    gtpool = pool("gt", 2)
    vpool = pool("v", 2)
    xvpool = pool("xv", 2)
    opool = pool("o", 3)
    ppool = pool("p", 2)
    outpool = pool("outs", 3)

    for ch in range(n_chunks):
        r0 = ch * CHUNK
        x_f = xpool.tile([128, N_RT, 256], F32, tag="xf")
        xT = xtpool.tile([128, 2, CHUNK], BF16, tag="xt")
        GT = gtpool.tile([128, 2, CHUNK], BF16, tag="gt")
        xb_all = vpool.tile([128, N_RT, 256], BF16, tag="xball")
        xv_b = xvpool.tile([1, CHUNK], BF16, tag="xvb")

        # load x, cast to bf16 (kept for the Y stage), transpose on PE
        for rt in range(N_RT):
            rr = r0 + rt * 128
            nc.sync.dma_start(out=x_f[:, rt, :], in_=x_d[rr:rr + 128, :])
            nc.vector.tensor_copy(xb_all[:, rt, :], x_f[:, rt, :])
            tpx = ptx.tile([128, 2, 128], BF16, tag="tx")
            for cc in range(2):
                nc.tensor.transpose(tpx[:, cc, :],
                                    xb_all[:, rt, cc * 128:(cc + 1) * 128],
                                    ident_b)
            nc.any.tensor_copy(xT[:, :, rt * 128:(rt + 1) * 128], tpx)

        # G^T = A^T x^T (a on partitions), xv = x . wv, in 512-col blocks
        for blk in range(CHUNK // 512):
            cs = blk * 512
            for ac in range(2):
                pg = pgv.tile([128, 512], F32, tag="g")
                for cc in range(2):
                    nc.tensor.matmul(pg, A_b[:, cc, ac * 128:(ac + 1) * 128],
                                     xT[:, cc, cs:cs + 512],
                                     start=(cc == 0), stop=(cc == 1))
                nc.any.tensor_copy(GT[:, ac, cs:cs + 512], pg)
            if use_bias:
                pxv = pgv.tile([128, 512], F32, tag="g")
                for cc in range(2):
                    nc.tensor.matmul(pxv[0:1, :], wv_b[:, cc:cc + 1],
                                     xT[:, cc, cs:cs + 512],
                                     start=(cc == 0), stop=(cc == 1))
                nc.any.tensor_copy(xv_b[0:1, cs:cs + 512], pxv[0:1, :])

        # attention, TRANSPOSED scores, 4 row-tiles per PSUM super-tile:
        # scT[j, i] = x_j . G_i per 128-row window; diagonal 64x64 blocks
        # are the pairs, off-diagonal cross-pair garbage. All 4 windows'
        # matmuls form ONE psum accumulation group (one bank). exp() runs
        # on the diagonal blocks of all 4 windows in one ACT op per half,
        # into a zeroed bf16 tile. 1/rowsum folds into the final residual.
        for sg in range(N_RT // 4):
            scT4 = psc.tile([128, 4, 128], F32, tag="sc")
            nmm = 12 if use_bias else 8
            mi = 0
            for r in range(4):
                rt = sg * 4 + r
                ir = rt * 128
                for ac in range(2):
                    nc.tensor.matmul(scT4[:, r, :], xT[:, ac, ir:ir + 128],
                                     GT[:, ac, ir:ir + 128],
                                     start=(mi == 0), stop=(mi == nmm - 1))
                    mi += 1
                if use_bias:
                    nc.tensor.matmul(scT4[:, r, :], xv_b[0:1, ir:ir + 128],
                                     ones_b, start=False, stop=(mi == nmm - 1))
                    mi += 1
            expT4 = expool.tile([128, 4, 128], BF16, tag="expt")
            nc.gpsimd.memset(expT4, 0.0)
            for il in range(2):
                dg = slice(il * 64, (il + 1) * 64)
                cg = slice(il * 64, il * 64 + 64)
                nc.scalar.activation(expT4[dg, :, cg], scT4[dg, :, cg],
                                     mybir.ActivationFunctionType.Exp)
            for r in range(4):
                rt = sg * 4 + r
                prs = ppo.tile([128, 1], F32, tag="po")
                nc.tensor.matmul(prs, expT4[:, r, :], onescol_b,
                                 start=True, stop=True)
                rrs_col = rspool.tile([128, 1], F32, tag="rrs")
                nc.vector.reciprocal(rrs_col, prs)
                # Y[c, i] = sum_j x[j, c] p^T[j, i]  (V projection fused
                # into U = Wv @ Wo on the output side)
                pou = pbig.tile([128, 256], F32, tag="m")
                for cc in range(2):
                    nc.tensor.matmul(pou[:, cc * 128:(cc + 1) * 128],
                                     xb_all[:, rt, cc * 128:(cc + 1) * 128],
                                     expT4[:, r, :], start=True, stop=True)
                Y_sb = opool.tile([128, 2, 128], BF16, tag="out_t")
                nc.any.tensor_copy(Y_sb, pou.rearrange("p (c i) -> p c i", c=2))
                # final projection, normalization, residual
                pO = ppo.tile([128, 256], F32, tag="po")
                nc.tensor.matmul(pO, Y_sb[:, 0, :], U_b[:, 0, :],
                                 start=True, stop=False)
                nc.tensor.matmul(pO, Y_sb[:, 1, :], U_b[:, 1, :],
                                 start=False, stop=True)
                o_sb = outpool.tile([128, 256], F32, tag="osb")
                nc.vector.scalar_tensor_tensor(
                    o_sb, pO, rrs_col, x_f[:, rt, :],
                    op0=mybir.AluOpType.mult, op1=mybir.AluOpType.add)
                if use_bias:
                    nc.vector.tensor_add(o_sb, o_sb, bo2_bc)
                rr = r0 + rt * 128
                nc.sync.dma_start(out=out_d[rr:rr + 128, :], in_=o_sb)



def build(n_chunks=RPC // CHUNK, use_bias=True):
    nc = bacc.Bacc("TRN2", target_bir_lowering=False, debug=False)
    rows = n_chunks * CHUNK
    x_d = nc.declare_dram_parameter("x", [rows, C], F32, isOutput=False)
    w_d = {n: nc.declare_dram_parameter(n, [C, C], F32, isOutput=False)
           for n in ("Wq", "Wk", "Wv", "Wo")}
    b_d = {n: nc.declare_dram_parameter(n, [C], F32, isOutput=False)
           for n in ("bq", "bk", "bv", "bo")}
    out_d = nc.declare_dram_parameter("out", [rows, C], F32, isOutput=True)
    from contextlib import ExitStack
    with tile.TileContext(nc) as tc, ExitStack() as ctx:
        _build_body(nc, tc, x_d, w_d, b_d, out_d, n_chunks, ctx, use_bias)
    nc.compile()
    return nc


_NC = {}
TRACE = False
LAST_RESULT = None


def kernel(x, Wq, bq, Wk, bk, Wv, bv, Wo, bo):
    global LAST_RESULT
    use_bias = any(np.any(np.asarray(b)) for b in (bq, bk, bv, bo))
    if use_bias not in _NC:
        _NC[use_bias] = build(use_bias=use_bias)
    nc_k = _NC[use_bias]
    from concourse.bass_utils import run_bass_kernel_spmd

    x = np.ascontiguousarray(np.asarray(x, dtype=np.float32))
    shared = {
        "Wq": np.ascontiguousarray(Wq, dtype=np.float32),
        "Wk": np.ascontiguousarray(Wk, dtype=np.float32),
        "Wv": np.ascontiguousarray(Wv, dtype=np.float32),
        "Wo": np.ascontiguousarray(Wo, dtype=np.float32),
        "bq": np.ascontiguousarray(bq, dtype=np.float32),
        "bk": np.ascontiguousarray(bk, dtype=np.float32),
        "bv": np.ascontiguousarray(bv, dtype=np.float32),
        "bo": np.ascontiguousarray(bo, dtype=np.float32),
    }
    in_maps = []
    for i in range(N_CORES):
        xs = np.ascontiguousarray(
            x[i * BPC:(i + 1) * BPC].reshape(RPC, C))
        in_maps.append({"x": xs, **shared})
    res = run_bass_kernel_spmd(nc_k, in_maps, core_ids=list(range(N_CORES)),
                               trace=TRACE)
    LAST_RESULT = res
    out = np.concatenate(
        [res.results[i]["out"].reshape(BPC, H, W, C) for i in range(N_CORES)],
        axis=0)
    return out


# revision 29
# speedup vs baseline: 1.1766x; 1.1417x over previous
"""Trainium2 Bass kernel for nn_AttentionBlock (b,h,w,c = 32,64,64,256).

out = x + (softmax_w(QK^T * s) @ V) @ Wo + bo   with Q/K/V = x@W* + b*
per-row attention over the w axis, batch-parallel over 8 NeuronCores.

Algebra used by the kernel (validated against the jax reference):
  scores*s = x A x^T + ones_i (x wv)^T   where A = (Wq Wk^T) s, wv = (Wk bq) s
  (the i-side bias term and the constant cancel inside softmax)
  out = attn@V@Wo + (bv@Wo + bo) + x
Compute dtype: bf16 operands into the PE array, fp32 PSUM accumulation,
fp32 residual add. Inputs/outputs stay fp32.
"""

import os
import sys

for _p in ("/opt/trn_rl_repo", os.path.expanduser("~/.axon_site/_ro/trn_rl_repo")):
    if os.path.isdir(_p) and _p not in sys.path:
        sys.path.append(_p)

import numpy as np

import concourse.bass as bass
import concourse.mybir as mybir
import concourse.tile as tile
from concourse import bacc
from concourse.masks import make_identity

N_CORES = 8
B, H, W, C = 32, 64, 64, 256
BPC = B // N_CORES            # batch images per core
RPC = BPC * H * W             # rows per core = 16384
SCALE = 1.0 / (C * np.sqrt(0.5) * np.sqrt(C))   # folded softmax scale

F32 = mybir.dt.float32
BF16 = mybir.dt.bfloat16

CHUNK = 2048                  # rows per chunk (32 attention pairs)
N_RT = CHUNK // 128           # row-tiles per chunk
N_PAIR = CHUNK // W           # pairs per chunk
N_GRP = N_PAIR // 8           # 8-pair score groups per chunk


def _build_body(nc, tc, x_d, w_d, b_d, out_d, n_chunks, ctx, use_bias):
    """Emit the kernel body. w_d/b_d: dicts of weight/bias dram handles."""

    def pool(name, bufs, space="SBUF"):
        kw = {} if space == "SBUF" else {"space": bass.MemorySpace.PSUM}
        return ctx.enter_context(tc.tile_pool(name=name, bufs=bufs, **kw))

    const = pool("const", 1)
    wtmp = pool("wtmp", 2)
    ptx = pool("ptx", 1, "PSUM")      # [128,128] bf16: transposes
    pbig = pool("pbig", 2, "PSUM")    # [128,256] f32: V, oU, A
    psc = pool("psc", 2, "PSUM")      # [128,4,128] f32: scores^T super-tiles
    ppo = pool("ppo", 2, "PSUM")      # [128,256] f32: O projection
    pgv = pool("pgv", 1, "PSUM")      # [128,512] f32: G blocks, xv

    # ---------------- preamble: constants & weight prep ----------------
    ident_b = const.tile([128, 128], BF16, tag="identb")
    make_identity(nc, ident_b)
    ones_b = const.tile([1, 128], BF16, tag="ones")
    nc.vector.memset(ones_b, 1.0)
    onescol_b = const.tile([128, 1], BF16, tag="onescol")
    nc.vector.memset(onescol_b, 1.0)

    # transposed Wq/Wk/Wv: WT[:, dc, c] = W[c, 128*dc + d]
    WqT = const.tile([128, 2, 256], BF16, tag="wqt")
    WkT = const.tile([128, 2, 256], BF16, tag="wkt")
    WvT = const.tile([128, 2, 256], BF16, tag="wvt")
    Wo_b = const.tile([128, 2, 256], BF16, tag="wob")
    for wname, wt in (("Wq", WqT), ("Wk", WkT), ("Wv", WvT)):
        for cc in range(2):
            wrow = wtmp.tile([128, 256], F32, tag="wrow")
            nc.sync.dma_start(out=wrow, in_=w_d[wname][cc * 128:(cc + 1) * 128, :])
            wrow_b = wtmp.tile([128, 256], BF16, tag="wrowb")
            nc.vector.tensor_copy(wrow_b, wrow)
            for dc in range(2):
                tp = ptx.tile([128, 128], BF16, tag="tx")
                nc.tensor.transpose(tp, wrow_b[:, dc * 128:(dc + 1) * 128], ident_b)
                nc.any.tensor_copy(wt[:, dc, cc * 128:(cc + 1) * 128], tp)
    for wname, wt in (("Wo", Wo_b),):
        for cc in range(2):
            wrow = wtmp.tile([128, 256], F32, tag="wrow")
            nc.sync.dma_start(out=wrow, in_=w_d[wname][cc * 128:(cc + 1) * 128, :])
            nc.any.tensor_copy(wt[:, cc, :], wrow)

    # A[c, a] = s * sum_d Wq[c, d] Wk[a, d]
    A_b = const.tile([128, 2, 256], BF16, tag="ab")
    for cc in range(2):
        pa = pbig.tile([128, 256], F32, tag="m")
        for dc in range(2):
            nc.tensor.matmul(pa, WqT[:, dc, cc * 128:(cc + 1) * 128], WkT[:, dc, :],
                             start=(dc == 0), stop=(dc == 1))
        nc.any.tensor_scalar_mul(A_b[:, cc, :], pa, float(SCALE))

    # U[c, e] = sum_d Wv[c, d] Wo[d, e]  (fused V+output projection)
    U_b = const.tile([128, 2, 256], BF16, tag="ub")
    for cc in range(2):
        pu = pbig.tile([128, 256], F32, tag="m")
        for dc in range(2):
            nc.tensor.matmul(pu, WvT[:, dc, cc * 128:(cc + 1) * 128],
                             Wo_b[:, dc, :], start=(dc == 0), stop=(dc == 1))
        nc.any.tensor_copy(U_b[:, cc, :], pu)

    wv_b = bo2_b = None
    if use_bias:
        # bias columns
        bq_b = const.tile([128, 2], BF16, tag="bqb")
        bv_b = const.tile([128, 2], BF16, tag="bvb")
        for bname, bt in (("bq", bq_b), ("bv", bv_b)):
            bf = wtmp.tile([128, 2], F32, tag="bcol")
            for cc in range(2):
                nc.sync.dma_start(out=bf[:, cc:cc + 1],
                                  in_=b_d[bname][cc * 128:(cc + 1) * 128].unsqueeze(1))
            nc.any.tensor_copy(bt, bf)

        # wv[c] = s * sum_d Wk[c, d] bq[d]
        wv_b = const.tile([128, 2], BF16, tag="wvvec")
        for cc in range(2):
            pw = psc.tile([128, 128], F32, tag="sc")
            for dc in range(2):
                nc.tensor.matmul(pw[:, 0:1], WkT[:, dc, cc * 128:(cc + 1) * 128],
                                 bq_b[:, dc:dc + 1], start=(dc == 0), stop=(dc == 1))
            nc.any.tensor_scalar_mul(wv_b[:, cc:cc + 1], pw[:, 0:1], float(SCALE))

        # bo2[e] = bv @ Wo + bo  (fused output bias, bf16 [1,256] row)
        bo_f = wtmp.tile([1, 256], F32, tag="borow")
        nc.sync.dma_start(out=bo_f, in_=b_d["bo"][:].unsqueeze(0))
        pb = pbig.tile([128, 256], F32, tag="m")
        for cc in range(2):
            nc.tensor.matmul(pb[0:1, :], bv_b[:, cc:cc + 1], Wo_b[:, cc, :],
                             start=(cc == 0), stop=(cc == 1))
        bo2_f = wtmp.tile([1, 256], F32, tag="bo2row")
        nc.vector.tensor_add(bo2_f, pb[0:1, :], bo_f)
        bo2_dram = nc.dram_tensor("bo2_bounce", [256], F32)
        nc.sync.dma_start(out=bo2_dram[:].unsqueeze(0), in_=bo2_f)
        bo2_bc = const.tile([128, 256], F32, tag="bo2bc")
        bo2_src = bass.AP(tensor=bo2_dram, offset=0,
                          ap=[[0, 128], [1, 256]])
        nc.sync.dma_start(out=bo2_bc, in_=bo2_src)

    # ---------------- main loop ----------------
    xpool = pool("x", 2)
    xbpool = pool("xb", 3)
    expool = pool("expt", 3)
    rspool = pool("rs", 3)
    xtpool = pool("xt", 2)
    gtpool = pool("gt", 2)
    vpool = pool("v", 2)
    xvpool = pool("xv", 2)
    opool = pool("o", 3)
    ppool = pool("p", 2)
    outpool = pool("outs", 3)

    for ch in range(n_chunks):
        r0 = ch * CHUNK
        x_f = xpool.tile([128, N_RT, 256], F32, tag="xf")
        xT = xtpool.tile([128, 2, CHUNK], BF16, tag="xt")
        GT = gtpool.tile([128, 2, CHUNK], BF16, tag="gt")
        xb_all = vpool.tile([128, N_RT, 256], BF16, tag="xball")
        xv_b = xvpool.tile([1, CHUNK], BF16, tag="xvb")

        # load x, cast to bf16 (kept for the Y stage), transpose on PE
        for rt in range(N_RT):
            rr = r0 + rt * 128
            nc.sync.dma_start(out=x_f[:, rt, :], in_=x_d[rr:rr + 128, :])
            nc.vector.tensor_copy(xb_all[:, rt, :], x_f[:, rt, :])
            tpx = ptx.tile([128, 2, 128], BF16, tag="tx")
            for cc in range(2):
                nc.tensor.transpose(tpx[:, cc, :],
                                    xb_all[:, rt, cc * 128:(cc + 1) * 128],
                                    ident_b)
            nc.any.tensor_copy(xT[:, :, rt * 128:(rt + 1) * 128], tpx)

        # G^T = A^T x^T (a on partitions), xv = x . wv, in 512-col blocks
        for blk in range(CHUNK // 512):
            cs = blk * 512
            for ac in range(2):
                pg = pgv.tile([128, 512], F32, tag="g")
                for cc in range(2):
                    nc.tensor.matmul(pg, A_b[:, cc, ac * 128:(ac + 1) * 128],
                                     xT[:, cc, cs:cs + 512],
                                     start=(cc == 0), stop=(cc == 1))
                nc.any.tensor_copy(GT[:, ac, cs:cs + 512], pg)
            if use_bias:
                pxv = pgv.tile([128, 512], F32, tag="g")
                for cc in range(2):
                    nc.tensor.matmul(pxv[0:1, :], wv_b[:, cc:cc + 1],
                                     xT[:, cc, cs:cs + 512],
                                     start=(cc == 0), stop=(cc == 1))
                nc.any.tensor_copy(xv_b[0:1, cs:cs + 512], pxv[0:1, :])

        # attention, TRANSPOSED scores, 4 row-tiles per PSUM super-tile:
        # scT[j, i] = x_j . G_i per 128-row window; diagonal 64x64 blocks
        # are the pairs, off-diagonal cross-pair garbage. All 4 windows'
        # matmuls form ONE psum accumulation group (one bank). exp() runs
        # on the diagonal blocks of all 4 windows in one ACT op per half,
        # into a zeroed bf16 tile. 1/rowsum folds into the final residual.
        for sg in range(N_RT // 4):
            scT4 = psc.tile([128, 4, 128], F32, tag="sc")
            nmm = 12 if use_bias else 8
            mi = 0
            for r in range(4):
                rt = sg * 4 + r
                ir = rt * 128
                for ac in range(2):
                    nc.tensor.matmul(scT4[:, r, :], xT[:, ac, ir:ir + 128],
                                     GT[:, ac, ir:ir + 128],
                                     start=(mi == 0), stop=(mi == nmm - 1))
                    mi += 1
                if use_bias:
                    nc.tensor.matmul(scT4[:, r, :], xv_b[0:1, ir:ir + 128],
                                     ones_b, start=False, stop=(mi == nmm - 1))
                    mi += 1
            expT4 = expool.tile([128, 4, 128], BF16, tag="expt")
            nc.gpsimd.memset(expT4, 0.0)
            for il in range(2):
                dg = slice(il * 64, (il + 1) * 64)
                cg = slice(il * 64, il * 64 + 64)
                nc.scalar.activation(expT4[dg, :, cg], scT4[dg, :, cg],
                                     mybir.ActivationFunctionType.Exp)
            for r in range(4):
                rt = sg * 4 + r
                prs = ppo.tile([128, 1], F32, tag="po")
                nc.tensor.matmul(prs, expT4[:, r, :], onescol_b,
                                 start=True, stop=True)
                rrs_col = rspool.tile([128, 1], F32, tag="rrs")
                nc.vector.reciprocal(rrs_col, prs)
                # Y[c, i] = sum_j x[j, c] p^T[j, i]  (V projection fused
                # into U = Wv @ Wo on the output side)
                pou = pbig.tile([128, 256], F32, tag="m")
                for cc in range(2):
                    nc.tensor.matmul(pou[:, cc * 128:(cc + 1) * 128],
                                     xb_all[:, rt, cc * 128:(cc + 1) * 128],
                                     expT4[:, r, :], start=True, stop=True)
                Y_sb = opool.tile([128, 2, 128], BF16, tag="out_t")
                nc.any.tensor_copy(Y_sb, pou.rearrange("p (c i) -> p c i", c=2))
                # final projection, normalization, residual
                pO = ppo.tile([128, 256], F32, tag="po")
                nc.tensor.matmul(pO, Y_sb[:, 0, :], U_b[:, 0, :],
                                 start=True, stop=False)
                nc.tensor.matmul(pO, Y_sb[:, 1, :], U_b[:, 1, :],
                                 start=False, stop=True)
                o_sb = outpool.tile([128, 256], F32, tag="osb")
                nc.vector.scalar_tensor_tensor(
                    o_sb, pO, rrs_col, x_f[:, rt, :],
                    op0=mybir.AluOpType.mult, op1=mybir.AluOpType.add)
                if use_bias:
                    nc.vector.tensor_add(o_sb, o_sb, bo2_bc)
                rr = r0 + rt * 128
                nc.sync.dma_start(out=out_d[rr:rr + 128, :], in_=o_sb)



def build(n_chunks=RPC // CHUNK, use_bias=True):
    nc = bacc.Bacc("TRN2", target_bir_lowering=False, debug=False)
    rows = n_chunks * CHUNK
    x_d = nc.declare_dram_parameter("x", [rows, C], F32, isOutput=False)
    w_d = {n: nc.declare_dram_parameter(n, [C, C], F32, isOutput=False)
           for n in ("Wq", "Wk", "Wv", "Wo")}
    b_d = {n: nc.declare_dram_parameter(n, [C], F32, isOutput=False)
           for n in ("bq", "bk", "bv", "bo")}
    out_d = nc.declare_dram_parameter("out", [rows, C], F32, isOutput=True)
    from contextlib import ExitStack
    with tile.TileContext(nc) as tc, ExitStack() as ctx:
        _build_body(nc, tc, x_d, w_d, b_d, out_d, n_chunks, ctx, use_bias)
    nc.compile()
    return nc


_NC = {}
TRACE = False
LAST_RESULT = None


def kernel(x, Wq, bq, Wk, bk, Wv, bv, Wo, bo):
    global LAST_RESULT
    use_bias = any(np.any(np.asarray(b)) for b in (bq, bk, bv, bo))
    if use_bias not in _NC:
        _NC[use_bias] = build(use_bias=use_bias)
    nc_k = _NC[use_bias]
    from concourse.bass_utils import run_bass_kernel_spmd

    x = np.ascontiguousarray(np.asarray(x, dtype=np.float32))
    shared = {
        "Wq": np.ascontiguousarray(Wq, dtype=np.float32),
        "Wk": np.ascontiguousarray(Wk, dtype=np.float32),
        "Wv": np.ascontiguousarray(Wv, dtype=np.float32),
        "Wo": np.ascontiguousarray(Wo, dtype=np.float32),
        "bq": np.ascontiguousarray(bq, dtype=np.float32),
        "bk": np.ascontiguousarray(bk, dtype=np.float32),
        "bv": np.ascontiguousarray(bv, dtype=np.float32),
        "bo": np.ascontiguousarray(bo, dtype=np.float32),
    }
    in_maps = []
    for i in range(N_CORES):
        xs = np.ascontiguousarray(
            x[i * BPC:(i + 1) * BPC].reshape(RPC, C))
        in_maps.append({"x": xs, **shared})
    res = run_bass_kernel_spmd(nc_k, in_maps, core_ids=list(range(N_CORES)),
                               trace=TRACE)
    LAST_RESULT = res
    out = np.concatenate(
        [res.results[i]["out"].reshape(BPC, H, W, C) for i in range(N_CORES)],
        axis=0)
    return out


# revision 30
# speedup vs baseline: 1.1878x; 1.0095x over previous
"""Trainium2 Bass kernel for nn_AttentionBlock (b,h,w,c = 32,64,64,256).

out = x + (softmax_w(QK^T * s) @ V) @ Wo + bo   with Q/K/V = x@W* + b*
per-row attention over the w axis, batch-parallel over 8 NeuronCores.

Algebra used by the kernel (validated against the jax reference):
  scores*s = x A x^T + ones_i (x wv)^T   where A = (Wq Wk^T) s, wv = (Wk bq) s
  (the i-side bias term and the constant cancel inside softmax)
  out = attn@V@Wo + (bv@Wo + bo) + x
Compute dtype: bf16 operands into the PE array, fp32 PSUM accumulation,
fp32 residual add. Inputs/outputs stay fp32.
"""

import os
import sys

for _p in ("/opt/trn_rl_repo", os.path.expanduser("~/.axon_site/_ro/trn_rl_repo")):
    if os.path.isdir(_p) and _p not in sys.path:
        sys.path.append(_p)

import numpy as np

import concourse.bass as bass
import concourse.mybir as mybir
import concourse.tile as tile
from concourse import bacc
from concourse.masks import make_identity

N_CORES = 8
B, H, W, C = 32, 64, 64, 256
BPC = B // N_CORES            # batch images per core
RPC = BPC * H * W             # rows per core = 16384
SCALE = 1.0 / (C * np.sqrt(0.5) * np.sqrt(C))   # folded softmax scale

F32 = mybir.dt.float32
BF16 = mybir.dt.bfloat16

CHUNK = 4096                  # rows per chunk (64 attention pairs)
N_RT = CHUNK // 128           # row-tiles per chunk
N_PAIR = CHUNK // W           # pairs per chunk
N_GRP = N_PAIR // 8           # 8-pair score groups per chunk


def _build_body(nc, tc, x_d, w_d, b_d, out_d, n_chunks, ctx, use_bias):
    """Emit the kernel body. w_d/b_d: dicts of weight/bias dram handles."""

    def pool(name, bufs, space="SBUF"):
        kw = {} if space == "SBUF" else {"space": bass.MemorySpace.PSUM}
        return ctx.enter_context(tc.tile_pool(name=name, bufs=bufs, **kw))

    const = pool("const", 1)
    wtmp = pool("wtmp", 2)
    ptx = pool("ptx", 1, "PSUM")      # [128,128] bf16: transposes
    pbig = pool("pbig", 2, "PSUM")    # [128,256] f32: V, oU, A
    psc = pool("psc", 2, "PSUM")      # [128,4,128] f32: scores^T super-tiles
    ppo = pool("ppo", 2, "PSUM")      # [128,256] f32: O projection
    pgv = pool("pgv", 1, "PSUM")      # [128,512] f32: G blocks, xv

    # ---------------- preamble: constants & weight prep ----------------
    ident_b = const.tile([128, 128], BF16, tag="identb")
    make_identity(nc, ident_b)
    ones_b = const.tile([1, 128], BF16, tag="ones")
    nc.vector.memset(ones_b, 1.0)
    onescol_b = const.tile([128, 1], BF16, tag="onescol")
    nc.vector.memset(onescol_b, 1.0)

    # transposed Wq/Wk/Wv: WT[:, dc, c] = W[c, 128*dc + d]
    WqT = const.tile([128, 2, 256], BF16, tag="wqt")
    WkT = const.tile([128, 2, 256], BF16, tag="wkt")
    WvT = const.tile([128, 2, 256], BF16, tag="wvt")
    Wo_b = const.tile([128, 2, 256], BF16, tag="wob")
    for wname, wt in (("Wq", WqT), ("Wk", WkT), ("Wv", WvT)):
        for cc in range(2):
            wrow = wtmp.tile([128, 256], F32, tag="wrow")
            nc.sync.dma_start(out=wrow, in_=w_d[wname][cc * 128:(cc + 1) * 128, :])
            wrow_b = wtmp.tile([128, 256], BF16, tag="wrowb")
            nc.vector.tensor_copy(wrow_b, wrow)
            for dc in range(2):
                tp = ptx.tile([128, 128], BF16, tag="tx")
                nc.tensor.transpose(tp, wrow_b[:, dc * 128:(dc + 1) * 128], ident_b)
                nc.any.tensor_copy(wt[:, dc, cc * 128:(cc + 1) * 128], tp)
    for wname, wt in (("Wo", Wo_b),):
        for cc in range(2):
            wrow = wtmp.tile([128, 256], F32, tag="wrow")
            nc.sync.dma_start(out=wrow, in_=w_d[wname][cc * 128:(cc + 1) * 128, :])
            nc.any.tensor_copy(wt[:, cc, :], wrow)

    # A[c, a] = s * sum_d Wq[c, d] Wk[a, d]
    A_b = const.tile([128, 2, 256], BF16, tag="ab")
    for cc in range(2):
        pa = pbig.tile([128, 256], F32, tag="m")
        for dc in range(2):
            nc.tensor.matmul(pa, WqT[:, dc, cc * 128:(cc + 1) * 128], WkT[:, dc, :],
                             start=(dc == 0), stop=(dc == 1))
        nc.any.tensor_scalar_mul(A_b[:, cc, :], pa, float(SCALE))

    # U[c, e] = sum_d Wv[c, d] Wo[d, e]  (fused V+output projection)
    U_b = const.tile([128, 2, 256], BF16, tag="ub")
    for cc in range(2):
        pu = pbig.tile([128, 256], F32, tag="m")
        for dc in range(2):
            nc.tensor.matmul(pu, WvT[:, dc, cc * 128:(cc + 1) * 128],
                             Wo_b[:, dc, :], start=(dc == 0), stop=(dc == 1))
        nc.any.tensor_copy(U_b[:, cc, :], pu)

    wv_b = bo2_b = None
    if use_bias:
        # bias columns
        bq_b = const.tile([128, 2], BF16, tag="bqb")
        bv_b = const.tile([128, 2], BF16, tag="bvb")
        for bname, bt in (("bq", bq_b), ("bv", bv_b)):
            bf = wtmp.tile([128, 2], F32, tag="bcol")
            for cc in range(2):
                nc.sync.dma_start(out=bf[:, cc:cc + 1],
                                  in_=b_d[bname][cc * 128:(cc + 1) * 128].unsqueeze(1))
            nc.any.tensor_copy(bt, bf)

        # wv[c] = s * sum_d Wk[c, d] bq[d]
        wv_b = const.tile([128, 2], BF16, tag="wvvec")
        for cc in range(2):
            pw = psc.tile([128, 128], F32, tag="sc")
            for dc in range(2):
                nc.tensor.matmul(pw[:, 0:1], WkT[:, dc, cc * 128:(cc + 1) * 128],
                                 bq_b[:, dc:dc + 1], start=(dc == 0), stop=(dc == 1))
            nc.any.tensor_scalar_mul(wv_b[:, cc:cc + 1], pw[:, 0:1], float(SCALE))

        # bo2[e] = bv @ Wo + bo  (fused output bias, bf16 [1,256] row)
        bo_f = wtmp.tile([1, 256], F32, tag="borow")
        nc.sync.dma_start(out=bo_f, in_=b_d["bo"][:].unsqueeze(0))
        pb = pbig.tile([128, 256], F32, tag="m")
        for cc in range(2):
            nc.tensor.matmul(pb[0:1, :], bv_b[:, cc:cc + 1], Wo_b[:, cc, :],
                             start=(cc == 0), stop=(cc == 1))
        bo2_f = wtmp.tile([1, 256], F32, tag="bo2row")
        nc.vector.tensor_add(bo2_f, pb[0:1, :], bo_f)
        bo2_dram = nc.dram_tensor("bo2_bounce", [256], F32)
        nc.sync.dma_start(out=bo2_dram[:].unsqueeze(0), in_=bo2_f)
        bo2_bc = const.tile([128, 256], F32, tag="bo2bc")
        bo2_src = bass.AP(tensor=bo2_dram, offset=0,
                          ap=[[0, 128], [1, 256]])
        nc.sync.dma_start(out=bo2_bc, in_=bo2_src)

    # ---------------- main loop ----------------
    xpool = pool("x", 2)
    xbpool = pool("xb", 3)
    expool = pool("expt", 3)
    rspool = pool("rs", 3)
    xtpool = pool("xt", 2)
    gtpool = pool("gt", 2)
    vpool = pool("v", 2)
    xvpool = pool("xv", 2)
    opool = pool("o", 3)
    ppool = pool("p", 2)
    outpool = pool("outs", 3)

    for ch in range(n_chunks):
        r0 = ch * CHUNK
        x_f = xpool.tile([128, N_RT, 256], F32, tag="xf")
        xT = xtpool.tile([128, 2, CHUNK], BF16, tag="xt")
        GT = gtpool.tile([128, 2, CHUNK], BF16, tag="gt")
        xb_all = vpool.tile([128, N_RT, 256], BF16, tag="xball")
        xv_b = xvpool.tile([1, CHUNK], BF16, tag="xvb")

        # load x, cast to bf16 (kept for the Y stage), transpose on PE
        for rt in range(N_RT):
            rr = r0 + rt * 128
            nc.sync.dma_start(out=x_f[:, rt, :], in_=x_d[rr:rr + 128, :])
            nc.vector.tensor_copy(xb_all[:, rt, :], x_f[:, rt, :])
            tpx = ptx.tile([128, 2, 128], BF16, tag="tx")
            for cc in range(2):
                nc.tensor.transpose(tpx[:, cc, :],
                                    xb_all[:, rt, cc * 128:(cc + 1) * 128],
                                    ident_b)
            nc.any.tensor_copy(xT[:, :, rt * 128:(rt + 1) * 128], tpx)

        # G^T = A^T x^T (a on partitions), xv = x . wv, in 512-col blocks
        for blk in range(CHUNK // 512):
            cs = blk * 512
            for ac in range(2):
                pg = pgv.tile([128, 512], F32, tag="g")
                for cc in range(2):
                    nc.tensor.matmul(pg, A_b[:, cc, ac * 128:(ac + 1) * 128],
                                     xT[:, cc, cs:cs + 512],
                                     start=(cc == 0), stop=(cc == 1))
                nc.any.tensor_copy(GT[:, ac, cs:cs + 512], pg)
            if use_bias:
                pxv = pgv.tile([128, 512], F32, tag="g")
                for cc in range(2):
                    nc.tensor.matmul(pxv[0:1, :], wv_b[:, cc:cc + 1],
                                     xT[:, cc, cs:cs + 512],
                                     start=(cc == 0), stop=(cc == 1))
                nc.any.tensor_copy(xv_b[0:1, cs:cs + 512], pxv[0:1, :])

        # attention, TRANSPOSED scores, 4 row-tiles per PSUM super-tile:
        # scT[j, i] = x_j . G_i per 128-row window; diagonal 64x64 blocks
        # are the pairs, off-diagonal cross-pair garbage. All 4 windows'
        # matmuls form ONE psum accumulation group (one bank). exp() runs
        # on the diagonal blocks of all 4 windows in one ACT op per half,
        # into a zeroed bf16 tile. 1/rowsum folds into the final residual.
        for sg in range(N_RT // 4):
            scT4 = psc.tile([128, 4, 128], F32, tag="sc")
            nmm = 12 if use_bias else 8
            mi = 0
            for r in range(4):
                rt = sg * 4 + r
                ir = rt * 128
                for ac in range(2):
                    nc.tensor.matmul(scT4[:, r, :], xT[:, ac, ir:ir + 128],
                                     GT[:, ac, ir:ir + 128],
                                     start=(mi == 0), stop=(mi == nmm - 1))
                    mi += 1
                if use_bias:
                    nc.tensor.matmul(scT4[:, r, :], xv_b[0:1, ir:ir + 128],
                                     ones_b, start=False, stop=(mi == nmm - 1))
                    mi += 1
            expT4 = expool.tile([128, 4, 128], BF16, tag="expt")
            nc.gpsimd.memset(expT4, 0.0)
            for il in range(2):
                dg = slice(il * 64, (il + 1) * 64)
                cg = slice(il * 64, il * 64 + 64)
                nc.scalar.activation(expT4[dg, :, cg], scT4[dg, :, cg],
                                     mybir.ActivationFunctionType.Exp)
            for r in range(4):
                rt = sg * 4 + r
                prs = ppo.tile([128, 1], F32, tag="po")
                nc.tensor.matmul(prs, expT4[:, r, :], onescol_b,
                                 start=True, stop=True)
                rrs_col = rspool.tile([128, 1], F32, tag="rrs")
                nc.vector.reciprocal(rrs_col, prs)
                # Y[c, i] = sum_j x[j, c] p^T[j, i]  (V projection fused
                # into U = Wv @ Wo on the output side)
                pou = pbig.tile([128, 256], F32, tag="m")
                for cc in range(2):
                    nc.tensor.matmul(pou[:, cc * 128:(cc + 1) * 128],
                                     xb_all[:, rt, cc * 128:(cc + 1) * 128],
                                     expT4[:, r, :], start=True, stop=True)
                Y_sb = opool.tile([128, 2, 128], BF16, tag="out_t")
                nc.any.tensor_copy(Y_sb, pou.rearrange("p (c i) -> p c i", c=2))
                # final projection, normalization, residual
                pO = ppo.tile([128, 256], F32, tag="po")
                nc.tensor.matmul(pO, Y_sb[:, 0, :], U_b[:, 0, :],
                                 start=True, stop=False)
                nc.tensor.matmul(pO, Y_sb[:, 1, :], U_b[:, 1, :],
                                 start=False, stop=True)
                o_sb = outpool.tile([128, 256], F32, tag="osb")
                nc.vector.scalar_tensor_tensor(
                    o_sb, pO, rrs_col, x_f[:, rt, :],
                    op0=mybir.AluOpType.mult, op1=mybir.AluOpType.add)
                if use_bias:
                    nc.vector.tensor_add(o_sb, o_sb, bo2_bc)
                rr = r0 + rt * 128
                nc.sync.dma_start(out=out_d[rr:rr + 128, :], in_=o_sb)



def build(n_chunks=RPC // CHUNK, use_bias=True):
    nc = bacc.Bacc("TRN2", target_bir_lowering=False, debug=False)
    rows = n_chunks * CHUNK
    x_d = nc.declare_dram_parameter("x", [rows, C], F32, isOutput=False)
    w_d = {n: nc.declare_dram_parameter(n, [C, C], F32, isOutput=False)
           for n in ("Wq", "Wk", "Wv", "Wo")}
    b_d = {n: nc.declare_dram_parameter(n, [C], F32, isOutput=False)
           for n in ("bq", "bk", "bv", "bo")}
    out_d = nc.declare_dram_parameter("out", [rows, C], F32, isOutput=True)
    from contextlib import ExitStack
    with tile.TileContext(nc) as tc, ExitStack() as ctx:
        _build_body(nc, tc, x_d, w_d, b_d, out_d, n_chunks, ctx, use_bias)
    nc.compile()
    return nc


_NC = {}
TRACE = False
LAST_RESULT = None


def kernel(x, Wq, bq, Wk, bk, Wv, bv, Wo, bo):
    global LAST_RESULT
    use_bias = any(np.any(np.asarray(b)) for b in (bq, bk, bv, bo))
    if use_bias not in _NC:
        _NC[use_bias] = build(use_bias=use_bias)
    nc_k = _NC[use_bias]
    from concourse.bass_utils import run_bass_kernel_spmd

    x = np.ascontiguousarray(np.asarray(x, dtype=np.float32))
    shared = {
        "Wq": np.ascontiguousarray(Wq, dtype=np.float32),
        "Wk": np.ascontiguousarray(Wk, dtype=np.float32),
        "Wv": np.ascontiguousarray(Wv, dtype=np.float32),
        "Wo": np.ascontiguousarray(Wo, dtype=np.float32),
        "bq": np.ascontiguousarray(bq, dtype=np.float32),
        "bk": np.ascontiguousarray(bk, dtype=np.float32),
        "bv": np.ascontiguousarray(bv, dtype=np.float32),
        "bo": np.ascontiguousarray(bo, dtype=np.float32),
    }
    in_maps = []
    for i in range(N_CORES):
        xs = np.ascontiguousarray(
            x[i * BPC:(i + 1) * BPC].reshape(RPC, C))
        in_maps.append({"x": xs, **shared})
    res = run_bass_kernel_spmd(nc_k, in_maps, core_ids=list(range(N_CORES)),
                               trace=TRACE)
    LAST_RESULT = res
    out = np.concatenate(
        [res.results[i]["out"].reshape(BPC, H, W, C) for i in range(N_CORES)],
        axis=0)
    return out
